# revision 9
# baseline (speedup 1.0000x reference)
"""Multi-head attention (B=2, N=2048, C=1024, H=16, D=64) on 8 Trainium2 cores.

Sharding: core c handles batch b=c//4 and heads [4r, 4r+4) where r=c%4.
After per-head attention, AllToAll collectives redistribute the attention
output from head-sharded to sequence-sharded; core g computes the output
projection for rows [g*256, (g+1)*256) of both batches.

Key design points vs the naive version:
- q/k are computed DIRECTLY TRANSPOSED ([d, n] layout, two heads stacked per
  128-partition tile) by using the weight matrix as the matmul's stationary
  operand, eliminating all PE transposes.
- LayerNorm stats (mean, mean-square) are computed by PE matmuls against a
  small 1/64 block-selector matrix; rstd = exp(-0.5*ln(var+eps)) so the
  scalar engine stays on the single natural_log_exp table set all kernel.
- Normalization is applied with two bf16 DVE tensor_tensor ops against
  DMA-broadcast per-column rows.
- Stage B softmax exp is split between the scalar engine (true exp) and the
  vector engine (Schraudolph exp2: bits = round(s*128/ln2 + 16256) written as
  int16 and bit-viewed as bf16). Split is per (pair, ih, head) so each
  softmax sum uses one engine consistently.
"""
import os
import numpy as np

B, N, C = 2, 2048, 1024
H, D = 16, 64
LN_EPS = 1e-6
N_CORES = 8
IH = 1024        # i-half width in the attention stage
NCH = 4          # stage-A n-chunks (512 each)

# Schraudolph exp2 constants (rounding confirmed on HW): for bf16 bits,
# bits16 = round(s * (2^7/(8*ln2)) + 127*2^7);  s in [-64, 64] -> safe.
EXP_A = float(128.0 / np.log(2.0) * 0.125)
EXP_B = float(127.0 * 128.0)

# (pair, ih, hp) units whose exp runs on the DVE (full tile), plus one unit
# split by i-column half (icc0 -> ACT, icc1 -> DVE).
DVE_FULL = {(0, 0, 1), (0, 1, 1), (1, 0, 1)}
DVE_SPLIT = (1, 1, 1)

_CACHE = {}


def _install_trace_shim():
    """Recreate the missing antenv.axon_hooks module so trace=True works."""
    import sys, types
    if "antenv.axon_hooks" in sys.modules:
        return
    try:
        import antenv
        mod = types.ModuleType("antenv.axon_hooks")
        mod._hook = None
        mod.set_axon_ntff_profile_hook = lambda h: setattr(mod, "_hook", h)
        mod.get_axon_ntff_profile_hook = lambda: mod._hook
        sys.modules["antenv.axon_hooks"] = mod
        antenv.axon_hooks = mod
        from trn_agent_boot.trn_boot import _ntff_profile_via_ctypes
        mod._hook = _ntff_profile_via_ctypes("/opt/axon/libaxon_pjrt.so")
    except Exception:
        pass


def _build(general):
    """general=False: q/k LN scale==1 and bias==0 (the fast path).
    general=True: adds a per-partition scale/bias pass and keeps all exp on
    the scalar engine (safe input range)."""
    import concourse.bacc as bacc
    import concourse.bass as bass
    import concourse.tile as tile
    from concourse import mybir
    from contextlib import ExitStack

    f32 = mybir.dt.float32
    bf16 = mybir.dt.bfloat16
    i16 = mybir.dt.int16
    AF = mybir.ActivationFunctionType
    OP = mybir.AluOpType

    AP = bass.AP
    nc = bacc.Bacc("TRN2", target_bir_lowering=False, debug=False,
                   num_devices=N_CORES)

    # ---- DRAM I/O ----
    xT_d = nc.dram_tensor("xT", [C, N], bf16, kind="ExternalInput")
    wq_d = nc.dram_tensor("wq", [C, 2, 128], bf16, kind="ExternalInput")
    wk_d = nc.dram_tensor("wk", [C, 2, 128], bf16, kind="ExternalInput")
    wv_d = nc.dram_tensor("wv", [C, 256], bf16, kind="ExternalInput")
    wproj_d = nc.dram_tensor("wproj", [C, C], bf16, kind="ExternalInput")
    bq_d = nc.dram_tensor("bq", [2, 128], f32, kind="ExternalInput")
    bk_d = nc.dram_tensor("bk", [2, 128], f32, kind="ExternalInput")
    bv_d = nc.dram_tensor("bv", [256], f32, kind="ExternalInput")
    bproj_d = nc.dram_tensor("bproj", [C], f32, kind="ExternalInput")
    L_d = nc.dram_tensor("lnL", [128, 2], bf16, kind="ExternalInput")
    # general path: per-partition LN scale/bias for q,k ([2 pairs, 128])
    gq_d = nc.dram_tensor("gq", [2, 128], f32, kind="ExternalInput")
    gk_d = nc.dram_tensor("gk", [2, 128], f32, kind="ExternalInput")
    hq_d = nc.dram_tensor("hq", [2, 128], f32, kind="ExternalInput")
    hk_d = nc.dram_tensor("hk", [2, 128], f32, kind="ExternalInput")
    out_d = nc.dram_tensor("out_part", [B, 256, C], f32, kind="ExternalOutput")

    # DRAM scratch
    stat_d = nc.dram_tensor("stat_scratch", [4, 2, 2, N], f32).ap()   # t, kind, head, n
    rm_d = nc.dram_tensor("rm_scratch", [4, 2, 2, N], bf16).ap()      # t, {r,m}, head, n
    z_d = nc.dram_tensor("z_scratch", [8, IH], f32).ap()
    zr_d = nc.dram_tensor("zr_scratch", [8, IH], bf16).ap()

    def bcast(dram_handle, n_parts, free):
        ap = dram_handle.ap()
        return AP(tensor=ap.tensor, offset=0, ap=[[0, n_parts], [1, free]])

    groups = [[0, 1, 2, 3, 4, 5, 6, 7]]

    with tile.TileContext(nc) as tc:
        with ExitStack() as ctx:
            g = ctx.enter_context(tc.tile_pool(name="globals", bufs=1))
            dram = ctx.enter_context(tc.tile_pool(name="dram", bufs=1, space="DRAM"))

            # ---- weights / constants in ----
            wq_sb = g.tile([128, 2, 8, 128], bf16, tag="wq")
            wk_sb = g.tile([128, 2, 8, 128], bf16, tag="wk")
            wv_sb = g.tile([128, 8, 256], bf16, tag="wv")
            L_sb = g.tile([128, 2], bf16, tag="lnL")
            bq_sb = g.tile([128, 2], f32, tag="bq")
            bk_sb = g.tile([128, 2], f32, tag="bk")
            bv_bc = g.tile([128, 256], f32, tag="bv")
            bproj_bc = g.tile([128, C], f32, tag="bproj")
            nc.sync.dma_start(out=L_sb, in_=L_d.ap())
            eps_t = g.tile([128, 1], f32, tag="eps")
            nc.vector.memset(eps_t, LN_EPS)
            for p in range(2):
                nc.sync.dma_start(out=bq_sb[:, p:p + 1], in_=bq_d.ap()[p, :].rearrange("(x o) -> x o", o=1))
                nc.sync.dma_start(out=bk_sb[:, p:p + 1], in_=bk_d.ap()[p, :].rearrange("(x o) -> x o", o=1))
            if general:
                gq_sb = g.tile([128, 2], f32, tag="gq")
                gk_sb = g.tile([128, 2], f32, tag="gk")
                hq_sb = g.tile([128, 2], f32, tag="hq")
                hk_sb = g.tile([128, 2], f32, tag="hk")
                for p in range(2):
                    nc.sync.dma_start(out=gq_sb[:, p:p + 1], in_=gq_d.ap()[p, :].rearrange("(x o) -> x o", o=1))
                    nc.sync.dma_start(out=gk_sb[:, p:p + 1], in_=gk_d.ap()[p, :].rearrange("(x o) -> x o", o=1))
                    nc.sync.dma_start(out=hq_sb[:, p:p + 1], in_=hq_d.ap()[p, :].rearrange("(x o) -> x o", o=1))
                    nc.sync.dma_start(out=hk_sb[:, p:p + 1], in_=hk_d.ap()[p, :].rearrange("(x o) -> x o", o=1))
            nc.sync.dma_start(out=bv_bc, in_=bcast(bv_d, 128, 256))
            nc.sync.dma_start(out=bproj_bc, in_=bcast(bproj_d, 128, C))
            for kc in range(8):
                nc.sync.dma_start(out=wq_sb[:, :, kc, :],
                                  in_=wq_d.ap()[kc * 128:(kc + 1) * 128, :, :])
                nc.sync.dma_start(out=wk_sb[:, :, kc, :],
                                  in_=wk_d.ap()[kc * 128:(kc + 1) * 128, :, :])
                nc.sync.dma_start(out=wv_sb[:, kc, :],
                                  in_=wv_d.ap()[kc * 128:(kc + 1) * 128, :])

            # xT in, n-window major so compute can start early
            xT = g.tile([128, 8, N], bf16, tag="xT")
            xa = xT_d.ap()
            for nw in range(16):
                nc.sync.dma_start(
                    out=xT[:, :, nw * 128:(nw + 1) * 128],
                    in_=AP(tensor=xa.tensor, offset=nw * 128,
                           ap=[[N, 128], [128 * N, 8], [1, 128]]))

            # wproj prefetch (used in stage C)
            wp_sb = g.tile([128, 8, C], bf16, tag="wp_sb")
            for kc in range(8):
                nc.sync.dma_start(out=wp_sb[:, kc, :],
                                  in_=wproj_d.ap()[kc * 128:(kc + 1) * 128, :])

            # ---- persistent activations ----
            q2 = g.tile([128, 2, N], bf16, tag="q2")
            k2 = g.tile([128, 2, N], bf16, tag="k2")
            v_all = g.tile([128, 16, 4, D + 1], bf16, tag="v_all")
            ones_t = g.tile([128, 16, 4, 1], f32, tag="ones_t")
            nc.vector.memset(ones_t, 1.0)
            nc.vector.tensor_copy(out=v_all[:, :, :, D:D + 1], in_=ones_t)

            cc_in = [dram.tile([8, 128, 256], bf16, name=f"cc_in{p}") for p in range(2)]
            cc_out = [dram.tile([8, 128, 256], bf16, name=f"cc_out{p}") for p in range(2)]

            # ================= Stage A =================
            with ExitStack() as actx:
                sa = actx.enter_context(tc.tile_pool(name="stageA", bufs=2))
                sqp = actx.enter_context(tc.tile_pool(name="sq_pool", bufs=3))
                rmp = actx.enter_context(tc.tile_pool(name="rm_pool", bufs=2))
                stp = actx.enter_context(tc.tile_pool(name="stats", bufs=2))
                psQ = actx.enter_context(tc.tile_pool(name="psQ", bufs=2, space="PSUM"))
                psV = actx.enter_context(tc.tile_pool(name="psV", bufs=2, space="PSUM"))
                psS = actx.enter_context(tc.tile_pool(name="psS", bufs=1, space="PSUM"))

                # ---- v (all 4 heads, all nt) ----
                for nt in range(16):
                    ps_v = psV.tile([128, 256], f32, tag="ps_v", name=f"ps_v{nt}")
                    for kc in range(8):
                        nc.tensor.matmul(ps_v, xT[:, kc, nt * 128:(nt + 1) * 128],
                                         wv_sb[:, kc, :], start=(kc == 0), stop=(kc == 7))
                    nc.vector.tensor_tensor(
                        out=v_all[:, nt, :, 0:D],
                        in0=ps_v.rearrange("p (h d) -> p h d", h=4),
                        in1=bv_bc.rearrange("p (h d) -> p h d", h=4),
                        op=OP.add)

                # ---- q/k transposed tensors ----
                # tensors: (kind, pair): 0=q,1=k
                tensors = [(0, 0), (1, 0), (0, 1), (1, 1)]

                def tname(ti):
                    kind, pair = tensors[ti]
                    return f"{'qk'[kind]}{pair}"

                tmp_tiles = {}
                for ti, (kind, pair) in enumerate(tensors):
                    w_sb = wq_sb if kind == 0 else wk_sb
                    b_sb = bq_sb if kind == 0 else bk_sb
                    tmp = sa.tile([128, N], bf16, tag="qktmp", name=f"tmp{ti}")
                    tmp_tiles[ti] = tmp
                    mu_rows = stp.tile([2, 2, N], f32, tag="st_rows", name=f"strow{ti}")
                    for ch in range(NCH):
                        nsl = slice(ch * 512, (ch + 1) * 512)
                        ps_t = psQ.tile([128, 512], f32, tag="ps_t", name=f"ps_t{ti}_{ch}")
                        for kc in range(8):
                            nc.tensor.matmul(ps_t, w_sb[:, pair, kc, :],
                                             xT[:, kc, nsl],
                                             start=(kc == 0), stop=(kc == 7))
                        # evac with bias add (per-partition), fp32->bf16
                        nc.scalar.activation(out=tmp[:, nsl], in_=ps_t,
                                             func=AF.Identity,
                                             bias=b_sb[:, pair:pair + 1], scale=1.0)
                        sq = sqp.tile([128, 512], bf16, tag="sq", name=f"sq{ti}_{ch}")
                        nc.vector.tensor_tensor(out=sq, in0=tmp[:, nsl],
                                                in1=tmp[:, nsl], op=OP.mult)
                        ps_st = psS.tile([2, 512], f32, tag="st_raw", name=f"st_r{ti}_{ch}")
                        ps_st2 = psS.tile([2, 512], f32, tag="st_sq", name=f"st_s{ti}_{ch}")
                        nc.tensor.matmul(ps_st, L_sb, tmp[:, nsl], start=True, stop=True)
                        nc.tensor.matmul(ps_st2, L_sb, sq, start=True, stop=True)
                        # ACT cannot write DRAM; stage rows in SBUF then DMA.
                        nc.scalar.activation(out=mu_rows[:, 0, nsl], in_=ps_st, func=AF.Copy)
                        nc.scalar.activation(out=mu_rows[:, 1, nsl], in_=ps_st2, func=AF.Copy)

                    # stats -> DRAM -> transposed [128, 2, 16]
                    for kd in range(2):
                        nc.sync.dma_start(out=stat_d[ti, kd], in_=mu_rows[:, kd, :])
                    mu_t = stp.tile([128, 2, 16], f32, tag="mu_t", name=f"mu_t{ti}")
                    m2_t = stp.tile([128, 2, 16], f32, tag="m2_t", name=f"m2_t{ti}")
                    nc.sync.dma_start(out=mu_t, in_=stat_d[ti, 0].rearrange("h (p i) -> p h i", p=128))
                    nc.sync.dma_start(out=m2_t, in_=stat_d[ti, 1].rearrange("h (p i) -> p h i", p=128))
                    musq = stp.tile([128, 2, 16], f32, tag="musq", name=f"musq{ti}")
                    nc.vector.tensor_tensor(out=musq, in0=mu_t, in1=mu_t, op=OP.mult)
                    var = stp.tile([128, 2, 16], f32, tag="var", name=f"var{ti}")
                    nc.vector.tensor_tensor(out=var, in0=m2_t, in1=musq, op=OP.subtract)
                    lnv = stp.tile([128, 2, 16], f32, tag="lnv", name=f"lnv{ti}")
                    nc.scalar.activation(out=lnv, in_=var, func=AF.Ln, bias=eps_t)
                    rstd = stp.tile([128, 2, 16], f32, tag="rstd", name=f"rstd{ti}")
                    nc.scalar.activation(out=rstd, in_=lnv, func=AF.Exp, scale=-0.5)
                    mhat = stp.tile([128, 2, 16], f32, tag="mhat", name=f"mhat{ti}")
                    nc.vector.tensor_tensor(out=mhat, in0=mu_t, in1=rstd, op=OP.mult)
                    r_bf = stp.tile([128, 2, 16], bf16, tag="r_bf", name=f"r_bf{ti}")
                    m_bf = stp.tile([128, 2, 16], bf16, tag="m_bf", name=f"m_bf{ti}")
                    nc.vector.tensor_copy(out=r_bf, in_=rstd)
                    nc.vector.tensor_copy(out=m_bf, in_=mhat)
                    nc.sync.dma_start(out=rm_d[ti, 0].rearrange("h (p i) -> p h i", p=128), in_=r_bf)
                    nc.sync.dma_start(out=rm_d[ti, 1].rearrange("h (p i) -> p h i", p=128), in_=m_bf)
                    # broadcast rows across the two 64-partition head halves
                    r_sb = rmp.tile([128, N], bf16, tag="r_sb", name=f"r_sb{ti}")
                    m_sb = rmp.tile([128, N], bf16, tag="m_sb", name=f"m_sb{ti}")
                    for hh in range(2):
                        src_r = rm_d[ti, 0, hh]
                        src_m = rm_d[ti, 1, hh]
                        nc.sync.dma_start(
                            out=r_sb[hh * 64:(hh + 1) * 64, :],
                            in_=AP(tensor=src_r.tensor, offset=src_r.offset, ap=[[0, 64], [1, N]]))
                        nc.sync.dma_start(
                            out=m_sb[hh * 64:(hh + 1) * 64, :],
                            in_=AP(tensor=src_m.tensor, offset=src_m.offset, ap=[[0, 64], [1, N]]))
                    # normalize: q2 = tmp*r - m  (all bf16 SBUF)
                    dest = q2 if kind == 0 else k2
                    gg = (gq_sb if kind == 0 else gk_sb) if general else None
                    hh_b = (hq_sb if kind == 0 else hk_sb) if general else None
                    for ch in range(NCH):
                        nsl = slice(ch * 512, (ch + 1) * 512)
                        t1 = sqp.tile([128, 512], bf16, tag="t1", name=f"t1_{ti}_{ch}")
                        nc.vector.tensor_tensor(out=t1, in0=tmp[:, nsl], in1=r_sb[:, nsl],
                                                op=OP.mult)
                        if general:
                            t2 = sqp.tile([128, 512], bf16, tag="t2", name=f"t2_{ti}_{ch}")
                            nc.vector.tensor_tensor(out=t2, in0=t1, in1=m_sb[:, nsl],
                                                    op=OP.subtract)
                            nc.vector.tensor_scalar(
                                out=dest[:, pair, nsl], in0=t2,
                                scalar1=gg[:, pair:pair + 1], scalar2=hh_b[:, pair:pair + 1],
                                op0=OP.mult, op1=OP.add)
                        else:
                            nc.vector.tensor_tensor(out=dest[:, pair, nsl], in0=t1,
                                                    in1=m_sb[:, nsl], op=OP.subtract)

            # ================= Stage B: attention =================
            with ExitStack() as bctx:
                pss = bctx.enter_context(tc.tile_pool(name="psSc", bufs=1, space="PSUM"))
                pso = bctx.enter_context(tc.tile_pool(name="psO", bufs=1, space="PSUM"))
                ptp = bctx.enter_context(tc.tile_pool(name="pt_pool", bufs=6))
                nrm = bctx.enter_context(tc.tile_pool(name="nrm", bufs=3))

                for pair in range(2):
                    for ih in range(2):
                        ps_o = {}
                        for hp in range(2):
                            ps_o[hp] = pso.tile([65, IH], f32, tag=f"ps_o{hp}",
                                                name=f"ps_o{pair}_{ih}_{hp}")
                        for jt in range(16):
                            pts = {}
                            ps_s = {}
                            for hp in range(2):
                                ps_s[hp] = pss.tile([128, IH], f32, tag=f"ps_s{hp}",
                                                    name=f"ps_s{pair}_{ih}_{hp}_{jt}")
                            for icc in range(2):
                                for hp in range(2):
                                    po = hp * 64
                                    nc.tensor.matmul(
                                        ps_s[hp][:, icc * 512:(icc + 1) * 512],
                                        k2[po:po + 64, pair, jt * 128:(jt + 1) * 128],
                                        q2[po:po + 64, pair,
                                           ih * IH + icc * 512: ih * IH + (icc + 1) * 512],
                                        start=True, stop=True)
                            for hp in range(2):
                                pt = ptp.tile([128, IH], bf16, tag=f"pt{hp}",
                                              name=f"pt{pair}_{ih}_{hp}_{jt}")
                                unit = (pair, ih, hp)
                                if general:
                                    mode = "act"
                                elif unit in DVE_FULL:
                                    mode = "dve"
                                elif unit == DVE_SPLIT:
                                    mode = "split"
                                else:
                                    mode = "act"
                                if mode == "act":
                                    nc.scalar.activation(out=pt, in_=ps_s[hp],
                                                         func=AF.Exp, scale=0.125)
                                elif mode == "dve":
                                    nc.vector.tensor_scalar(
                                        out=pt.bitcast(i16), in0=ps_s[hp],
                                        scalar1=EXP_A, scalar2=EXP_B,
                                        op0=OP.mult, op1=OP.add)
                                else:  # split: icc0 on ACT, icc1 on DVE
                                    nc.scalar.activation(out=pt[:, 0:512],
                                                         in_=ps_s[hp][:, 0:512],
                                                         func=AF.Exp, scale=0.125)
                                    nc.vector.tensor_scalar(
                                        out=pt.bitcast(i16)[:, 512:1024],
                                        in0=ps_s[hp][:, 512:1024],
                                        scalar1=EXP_A, scalar2=EXP_B,
                                        op0=OP.mult, op1=OP.add)
                                pts[hp] = pt
                            for icc in range(2):
                                for hp in range(2):
                                    nc.tensor.matmul(
                                        ps_o[hp][:, icc * 512:(icc + 1) * 512],
                                        v_all[:, jt, 2 * pair + hp, :],
                                        pts[hp][:, icc * 512:(icc + 1) * 512],
                                        start=(jt == 0), stop=(jt == 15))

                        for hp in range(2):
                            h = 2 * pair + hp
                            slot = 2 * h + ih
                            # Z row -> recip -> broadcast [64, IH]
                            z_sb = nrm.tile([1, IH], f32, tag="z_sb", name=f"z{slot}")
                            nc.scalar.activation(out=z_sb, in_=ps_o[hp][64:65, :], func=AF.Copy)
                            nc.sync.dma_start(out=z_d[slot:slot + 1, :], in_=z_sb)
                            zt = nrm.tile([128, 8], f32, tag="zt", name=f"zt{slot}")
                            nc.sync.dma_start(out=zt,
                                              in_=z_d[slot, :].rearrange("(p t) -> p t", p=128))
                            rt = nrm.tile([128, 8], f32, tag="rt", name=f"rt{slot}")
                            nc.vector.reciprocal(out=rt, in_=zt)
                            rt_bf = nrm.tile([128, 8], bf16, tag="rt_bf", name=f"rtb{slot}")
                            nc.vector.tensor_copy(out=rt_bf, in_=rt)
                            nc.sync.dma_start(out=zr_d[slot, :].rearrange("(p t) -> p t", p=128),
                                              in_=rt_bf)
                            r64 = nrm.tile([64, IH], bf16, tag="r64", name=f"r64_{slot}")
                            zr_s = zr_d[slot, :]
                            nc.sync.dma_start(
                                out=r64,
                                in_=AP(tensor=zr_s.tensor, offset=zr_s.offset,
                                       ap=[[0, 64], [1, IH]]))
                            outT_t = nrm.tile([64, IH], bf16, tag="outT", name=f"oT{slot}")
                            nc.vector.tensor_tensor(out=outT_t, in0=ps_o[hp][0:64, :],
                                                    in1=r64, op=OP.mult)
                            nc.gpsimd.dma_start(
                                out=cc_in[pair][4 * ih:4 * ih + 4,
                                                hp * 64:(hp + 1) * 64, :]
                                    .rearrange("s d i -> d s i"),
                                in_=outT_t.rearrange("d (s i) -> d s i", s=4))

                    nc.gpsimd.collective_compute(
                        "AllToAll", mybir.AluOpType.bypass, replica_groups=groups,
                        ins=[cc_in[pair].opt()], outs=[cc_out[pair].opt()])

            # ================= Stage C: projection =================
            with ExitStack() as cctx:
                atp = cctx.enter_context(tc.tile_pool(name="at_pool", bufs=3))
                psP = cctx.enter_context(tc.tile_pool(name="psP", bufs=1, space="PSUM"))
                oup = cctx.enter_context(tc.tile_pool(name="out_pool", bufs=3))

                ps_list = {}
                for bb in range(B):
                    for mt in range(2):
                        for nk in range(2):
                            ps_p = psP.tile([128, 512], f32, tag=f"ps_p{bb}{mt}{nk}")
                            ps_list[(bb, mt, nk)] = ps_p
                kc_order = [0, 2, 4, 6, 1, 3, 5, 7]
                for ki, kc in enumerate(kc_order):
                    wp_t = wp_sb[:, kc, :]
                    for bb in range(B):
                        at_t = atp.tile([128, 256], bf16, tag="at_t")
                        for half, gh in enumerate((2 * kc, 2 * kc + 1)):
                            lh = gh % 4
                            nc.sync.dma_start(
                                out=at_t[half * 64:(half + 1) * 64, :],
                                in_=cc_out[lh // 2][4 * bb + gh // 4,
                                                    (lh % 2) * 64:(lh % 2 + 1) * 64, :])
                        for mt in range(2):
                            for nk in range(2):
                                nc.tensor.matmul(
                                    ps_list[(bb, mt, nk)],
                                    at_t[:, mt * 128:(mt + 1) * 128],
                                    wp_t[:, nk * 512:(nk + 1) * 512],
                                    start=(ki == 0), stop=(ki == 7))
                for bb in range(B):
                    for mt in range(2):
                        o_sb = oup.tile([128, C], f32, tag="o_sb")
                        for nk in range(2):
                            nc.vector.tensor_tensor(
                                out=o_sb[:, nk * 512:(nk + 1) * 512],
                                in0=ps_list[(bb, mt, nk)],
                                in1=bproj_bc[:, nk * 512:(nk + 1) * 512],
                                op=OP.add)
                        nc.sync.dma_start(
                            out=out_d.ap()[bb, mt * 128:(mt + 1) * 128, :], in_=o_sb)

    nc.compile()
    return nc


def kernel(**inputs):
    from concourse.bass_utils import run_bass_kernel_spmd
    import ml_dtypes

    trace = os.environ.get("KERNEL_TRACE", "0") == "1"
    if trace:
        _install_trace_shim()

    bf = ml_dtypes.bfloat16

    x = np.asarray(inputs["x"], dtype=np.float32)
    w_qkv = np.asarray(inputs["w_qkv"], dtype=np.float32)
    b_qkv = np.asarray(inputs["b_qkv"], dtype=np.float32)
    w_proj = np.asarray(inputs["w_proj"], dtype=np.float32)
    b_proj = np.asarray(inputs["b_proj"], dtype=np.float32)
    q_scale = np.asarray(inputs["q_scale"], dtype=np.float32)
    q_bias = np.asarray(inputs["q_bias"], dtype=np.float32)
    k_scale = np.asarray(inputs["k_scale"], dtype=np.float32)
    k_bias = np.asarray(inputs["k_bias"], dtype=np.float32)

    general = not (np.all(q_scale == 1.0) and np.all(k_scale == 1.0)
                   and np.all(q_bias == 0.0) and np.all(k_bias == 0.0))

    key = "nc_gen" if general else "nc_fast"
    if key not in _CACHE:
        _CACHE[key] = _build(general)
    nc = _CACHE[key]

    # stats selector: col0 sums head A (partitions 0-63), col1 head B; x1/64
    L = np.zeros((128, 2), dtype=np.float32)
    L[0:64, 0] = 1.0 / 64.0
    L[64:128, 1] = 1.0 / 64.0

    wproj_m = np.ascontiguousarray(w_proj.astype(bf))

    in_maps = []
    for c in range(N_CORES):
        b, r = divmod(c, 4)
        base = 4 * r * D                       # this core's head-col base (256 wide)
        wq = np.ascontiguousarray(
            w_qkv[:, 0 * C + base: 0 * C + base + 256].reshape(C, 2, 128).astype(bf))
        wk = np.ascontiguousarray(
            w_qkv[:, 1 * C + base: 1 * C + base + 256].reshape(C, 2, 128).astype(bf))
        wv = np.ascontiguousarray(w_qkv[:, 2 * C + base: 2 * C + base + 256].astype(bf))
        bq = np.ascontiguousarray(b_qkv[0 * C + base: 0 * C + base + 256].reshape(2, 128))
        bk = np.ascontiguousarray(b_qkv[1 * C + base: 1 * C + base + 256].reshape(2, 128))
        bv = np.ascontiguousarray(b_qkv[2 * C + base: 2 * C + base + 256])
        m = {
            "xT": np.ascontiguousarray(x[b].T.astype(bf)),
            "wq": wq, "wk": wk, "wv": wv, "wproj": wproj_m,
            "bq": bq, "bk": bk, "bv": bv, "bproj": b_proj,
            "lnL": np.ascontiguousarray(L.astype(bf)),
        }
        if general:
            m["gq"] = np.ascontiguousarray(np.tile(q_scale, 2).reshape(2, 128))
            m["gk"] = np.ascontiguousarray(np.tile(k_scale, 2).reshape(2, 128))
            m["hq"] = np.ascontiguousarray(np.tile(q_bias, 2).reshape(2, 128))
            m["hk"] = np.ascontiguousarray(np.tile(k_bias, 2).reshape(2, 128))
        else:
            z2 = np.zeros((2, 128), dtype=np.float32)
            m["gq"] = z2; m["gk"] = z2; m["hq"] = z2; m["hk"] = z2
        in_maps.append(m)

    res = run_bass_kernel_spmd(nc, in_maps, core_ids=list(range(N_CORES)),
                               trace=trace)
    _CACHE["last_result"] = res

    out = np.empty((B, N, C), dtype=np.float32)
    for c in range(N_CORES):
        out[:, c * 256:(c + 1) * 256, :] = res.results[c]["out_part"]
    return out
